# revision 1
# baseline (speedup 1.0000x reference)
"""Segment mean-pool (LocalPooling1D) Trainium2 Bass kernel.

x [32, 8192, 256] f32, x_pos [32, 65] sorted int32 boundaries -> y [32, 64, 256].
y[b, j] = mean(x[b, x_pos[b,j]:x_pos[b,j+1]]), empty segments -> 0.

Strategy: data-parallel over batch, 4 rows per core on 8 cores. The 0/1
segment-indicator ind[t, j] = (pos[j] <= t < pos[j+1]) for all 64 token-tiles
of a row is built in two wide DVE ops (stride-0 broadcast of pos along the
tile axis; S = (pos - p <= 128*ti) via fused scalar_tensor_tensor, then a
shifted subtract). Segment sums accumulate on the TensorEngine as
psum += ind_tile.T @ x_tile in fp32, with even/odd token-tiles packed into
separate PE column groups (concurrent sub-array matmuls) to halve the fp32
matmul wall time. Finally y = (psum_even + psum_odd) * 1/max(count, 1).
"""

import os
import sys

import numpy as np

sys.path.insert(0, "/opt/trn_rl_repo")

import concourse.bacc as bacc
import concourse.bass as bass
import concourse.tile as tile
from concourse import mybir
from concourse.bass_utils import run_bass_kernel_spmd

dt = mybir.dt
Alu = mybir.AluOpType

# Problem constants (hardcoded per harness contract).
B, T, C, P = 32, 8192, 256, 65
NSEG = P - 1
NCORES = 8
R = B // NCORES          # batch rows per core
TOK = 128                # tokens per matmul tile (K)
KTILES = T // TOK        # 64 matmul tiles per row

CFG = {
    "blk": int(os.environ.get("KB_BLK", "8")),            # token-tiles per x DMA
    "col_pack": os.environ.get("KB_COLPACK", "1") == "1", # even/odd PE col groups
    "x_bufs": int(os.environ.get("KB_XBUFS", "12")),
    "ind_bufs": int(os.environ.get("KB_INDBUFS", "2")),
    "psum_bufs": int(os.environ.get("KB_PSUMBUFS", "2")),
    "dual_dma": os.environ.get("KB_DUALDMA", "1") == "1",
}


def build_program(cfg=CFG):
    blk = cfg["blk"]
    nblk = KTILES // blk
    col_pack = cfg["col_pack"]

    nc = bacc.Bacc("TRN2", target_bir_lowering=False, debug=False)

    x_d = nc.dram_tensor("x", [R, T, C], dt.float32, kind="ExternalInput")
    pos_d = nc.dram_tensor("x_pos", [R, P], dt.int32, kind="ExternalInput")
    y_d = nc.dram_tensor("y", [R, NSEG, C], dt.float32, kind="ExternalOutput")

    with tile.TileContext(nc) as tc:
        with (
            tc.tile_pool(name="const", bufs=1) as constp,
            tc.tile_pool(name="xp", bufs=cfg["x_bufs"]) as xp,
            tc.tile_pool(name="indp", bufs=cfg["ind_bufs"]) as indp,
            tc.tile_pool(name="smallp", bufs=4) as smallp,
            tc.tile_pool(name="outp", bufs=2) as outp,
            tc.tile_pool(name="psp", bufs=cfg["psum_bufs"], space="PSUM") as psp,
        ):
            # 128*ti along the tile axis, const across partitions/segments.
            # Values <= 8064 are exact in f32, so iota directly in f32.
            tio_b = constp.tile([TOK, KTILES, P], dt.float32)
            nc.gpsimd.iota(tio_b[:], pattern=[[TOK, KTILES], [0, P]], base=0,
                           channel_multiplier=0, allow_small_or_imprecise_dtypes=True)
            # partition index p as a per-partition scalar.
            p_iota = constp.tile([TOK, 1], dt.float32)
            nc.gpsimd.iota(p_iota[:], pattern=[[1, 1]], base=0, channel_multiplier=1,
                           allow_small_or_imprecise_dtypes=True)

            for r in range(R):
                # pos row -> [1, 65] f32, broadcast to [128, 65].
                pos_row = smallp.tile([1, P], dt.int32)
                nc.gpsimd.dma_start(pos_row[:], pos_d[r : r + 1, :])
                posf_row = smallp.tile([1, P], dt.float32)
                nc.vector.tensor_copy(posf_row[:], pos_row[:])
                pos_b = smallp.tile([TOK, P], dt.float32)
                nc.gpsimd.partition_broadcast(pos_b[:], posf_row[:])

                # counts -> 1/max(cnt, 1), partition-major [64, 1].
                pos_lo = smallp.tile([NSEG, 1], dt.int32)
                pos_hi = smallp.tile([NSEG, 1], dt.int32)
                nc.gpsimd.dma_start(pos_lo[:], pos_d[r : r + 1, 0:NSEG].rearrange("one p -> p one"))
                nc.gpsimd.dma_start(pos_hi[:], pos_d[r : r + 1, 1:P].rearrange("one p -> p one"))
                cnt_f = smallp.tile([NSEG, 1], dt.float32)
                nc.vector.tensor_tensor(cnt_f[:], pos_hi[:], pos_lo[:], op=Alu.subtract)
                cntc = smallp.tile([NSEG, 1], dt.float32)
                nc.vector.tensor_scalar(cntc[:], cnt_f[:], 1.0, None, op0=Alu.max)
                recip = smallp.tile([NSEG, 1], dt.float32)
                nc.vector.reciprocal(recip[:], cntc[:])

                # S[p, ti, j] = (pos[j] <= 128*ti + p), one fused DVE op.
                S_all = indp.tile([TOK, KTILES, P], dt.float32, tag="sall")
                nc.vector.scalar_tensor_tensor(
                    S_all[:],
                    pos_b[:, None, :].broadcast_to((TOK, KTILES, P)),
                    p_iota[:],
                    tio_b[:],
                    op0=Alu.subtract,
                    op1=Alu.is_le,
                )
                # ind[p, ti, j] = S[p, ti, j] - S[p, ti, j+1]
                ind_all = indp.tile([TOK, KTILES, NSEG], dt.float32, tag="ind")
                nc.vector.tensor_tensor(
                    ind_all[:], S_all[:, :, 0:NSEG], S_all[:, :, 1:P], op=Alu.subtract
                )

                ps = psp.tile([2 * NSEG if col_pack else NSEG, C], dt.float32)
                xr = x_d[r].rearrange("(b k p) c -> b p k c", k=blk, p=TOK)
                for b in range(nblk):
                    xt = xp.tile([TOK, blk * C], dt.float32)
                    xt_v = xt[:].rearrange("p (k c) -> p k c", k=blk)
                    eng = nc.scalar if (cfg["dual_dma"] and b % 2) else nc.sync
                    eng.dma_start(xt_v, xr[b])
                    for k in range(blk):
                        ti = b * blk + k
                        rhs = xt[:, k * C : (k + 1) * C]
                        lhsT = ind_all[:, ti, :]
                        if col_pack:
                            half = ti % 2
                            nc.tensor.matmul(
                                ps[half * NSEG : (half + 1) * NSEG, :], lhsT, rhs,
                                start=(ti == half), stop=(ti == KTILES - 2 + half),
                                tile_position=(0, half * NSEG),
                                skip_group_check=True,
                            )
                        else:
                            nc.tensor.matmul(
                                ps[:], lhsT, rhs,
                                start=(ti == 0), stop=(ti == KTILES - 1),
                            )

                out_t = outp.tile([NSEG, C], dt.float32)
                if col_pack:
                    # DVE reads one PSUM operand per op: scale each half alone.
                    half_t = outp.tile([NSEG, C], dt.float32, tag="half")
                    nc.vector.tensor_scalar(
                        half_t[:], ps[NSEG : 2 * NSEG, :], recip[:], None, op0=Alu.mult
                    )
                    nc.vector.scalar_tensor_tensor(
                        out_t[:], ps[0:NSEG, :], recip[:], half_t[:],
                        op0=Alu.mult, op1=Alu.add,
                    )
                else:
                    nc.vector.tensor_scalar(out_t[:], ps[:], recip[:], None, op0=Alu.mult)
                nc.gpsimd.dma_start(y_d[r], out_t[:])

    nc.compile()
    return nc


_PROGRAM = None


def _get_program():
    global _PROGRAM
    if _PROGRAM is None:
        _PROGRAM = build_program()
    return _PROGRAM


def kernel(x, x_pos):
    x = np.ascontiguousarray(x, dtype=np.float32)
    x_pos = np.ascontiguousarray(x_pos, dtype=np.int32)
    nc = _get_program()
    in_maps = [
        {"x": x[c * R : (c + 1) * R], "x_pos": x_pos[c * R : (c + 1) * R]}
        for c in range(NCORES)
    ]
    res = run_bass_kernel_spmd(nc, in_maps, list(range(NCORES)))
    y = np.concatenate([res.results[c]["y"] for c in range(NCORES)], axis=0)
    return y.astype(np.float32)



# revision 5
# speedup vs baseline: 1.0005x; 1.0005x over previous
"""Segment mean-pool (LocalPooling1D) Trainium2 Bass kernel.

x [32, 8192, 256] f32, x_pos [32, 65] sorted int32 boundaries -> y [32, 64, 256].
y[b, j] = mean(x[b, x_pos[b,j]:x_pos[b,j+1]]), empty segments -> 0.

Strategy: data-parallel over batch, 4 rows per core on 8 cores. The kernel is
HBM-bound (33.55 MB of x per core; ~94 us at the 358 GB/s per-core cap), so the
structure keeps the DMA stream saturated end-to-end:

- x is streamed on THREE hardware DMA queues (sync/scalar/gpsimd engines,
  rotating per block) since one queue's descriptor-issue rate caps at
  ~175 GB/s; three queues saturate the 16 DMA engines (~358 GB/s).
- The 0/1 segment-indicator ind[t, j] is built per row in two wide DVE ops
  from stride-0 broadcast views only (no big gpsimd iota); pos is replicated
  across partitions with a tiny PE matmul (ones[1,128].T @ posf[1,260]) so the
  gpsimd queue is never blocked before its x-block issues.
- This makes ind[row 0] ready ~17 us in (vs ~43 us before), so TensorEngine
  segment-sum matmuls track the DMA stream closely instead of piling into a
  tail burst that starves the last x blocks via SBUF contention.
- Segment sums accumulate as psum += ind_tile.T @ x_tile in fp32 with even/odd
  token-tiles in separate PE column groups (concurrent sub-array matmuls).
  Finally y = psum * 1/max(count, 1), counts from a transposed pos load.
"""

import os
import sys

import numpy as np

sys.path.insert(0, "/opt/trn_rl_repo")

import concourse.bacc as bacc
import concourse.bass as bass
import concourse.tile as tile
from concourse import mybir
from concourse.bass_utils import run_bass_kernel_spmd

dt = mybir.dt
Alu = mybir.AluOpType

# Problem constants (hardcoded per harness contract).
B, T, C, P = 32, 8192, 256, 65
NSEG = P - 1
NCORES = 8
R = B // NCORES          # batch rows per core
TOK = 128                # tokens per matmul tile (K)
KTILES = T // TOK        # 64 matmul tiles per row

CFG = {
    "blk": int(os.environ.get("KB_BLK", "4")),            # token-tiles per x DMA
    "x_bufs": int(os.environ.get("KB_XBUFS", "20")),
    "ind_bufs": int(os.environ.get("KB_INDBUFS", "4")),
    "s_bufs": int(os.environ.get("KB_SBUFS", "2")),
    "psum_bufs": int(os.environ.get("KB_PSUMBUFS", "2")),
    "nq": int(os.environ.get("KB_NQ", "3")),              # x DMA queues
}


def build_program(cfg=CFG):
    blk = cfg["blk"]
    nblk = KTILES // blk

    nc = bacc.Bacc("TRN2", target_bir_lowering=False, debug=False)

    x_d = nc.dram_tensor("x", [R, T, C], dt.float32, kind="ExternalInput")
    pos_d = nc.dram_tensor("x_pos", [R, P], dt.int32, kind="ExternalInput")
    y_d = nc.dram_tensor("y", [R, NSEG, C], dt.float32, kind="ExternalOutput")

    with tile.TileContext(nc) as tc:
        with (
            tc.tile_pool(name="const", bufs=1) as constp,
            tc.tile_pool(name="xp", bufs=cfg["x_bufs"]) as xp,
            tc.tile_pool(name="sp", bufs=cfg["s_bufs"]) as sp,
            tc.tile_pool(name="indp", bufs=cfg["ind_bufs"]) as indp,
            tc.tile_pool(name="smallp", bufs=1) as smallp,
            tc.tile_pool(name="outp", bufs=2) as outp,
            tc.tile_pool(name="psp", bufs=cfg["psum_bufs"], space="PSUM") as psp,
            tc.tile_pool(name="pbp", bufs=1, space="PSUM") as pbp,
        ):
            # 128*ti along the tile axis ([128, KTILES], same on every
            # partition); values <= 8064 are exact in f32.
            tio_p = constp.tile([TOK, KTILES], dt.float32)
            nc.gpsimd.iota(tio_p[:], pattern=[[TOK, KTILES]], base=0,
                           channel_multiplier=0,
                           allow_small_or_imprecise_dtypes=True)
            # partition index p as a per-partition scalar.
            p_iota = constp.tile([TOK, 1], dt.float32)
            nc.gpsimd.iota(p_iota[:], pattern=[[1, 1]], base=0,
                           channel_multiplier=1,
                           allow_small_or_imprecise_dtypes=True)
            ones_row = constp.tile([1, TOK], dt.float32)
            nc.vector.memset(ones_row[:], 1.0)

            # pos rows -> [1, R*P] f32; tiny dependency-free loads issued on
            # gpsimd before its x-block issues (sync/scalar start x at once).
            pos_i = smallp.tile([1, R * P], dt.int32)
            nc.gpsimd.dma_start(
                pos_i[0:1, :].rearrange("one (r p) -> one r p", r=R),
                pos_d[:, :],
            )
            posf = smallp.tile([1, R * P], dt.float32)
            nc.vector.tensor_copy(posf[:], pos_i[:])
            # Replicate to all 128 partitions via PE: ones.T @ posf (exact).
            pos_b = pbp.tile([TOK, R * P], dt.float32)
            nc.tensor.matmul(pos_b[:], ones_row[:], posf[:], start=True,
                             stop=True)

            # counts -> 1/max(cnt, 1), partition-major [NSEG, R] from two
            # transposed pos loads (DVE operands must start at partition 0).
            pos_lo = smallp.tile([NSEG, R], dt.int32)
            pos_hi = smallp.tile([NSEG, R], dt.int32)
            nc.gpsimd.dma_start(pos_lo[:],
                                pos_d[:, 0:NSEG].rearrange("r p -> p r"))
            nc.gpsimd.dma_start(pos_hi[:],
                                pos_d[:, 1:P].rearrange("r p -> p r"))
            cnt_f = smallp.tile([NSEG, R], dt.float32)
            nc.vector.tensor_tensor(cnt_f[:], pos_hi[:], pos_lo[:],
                                    op=Alu.subtract)
            cntc = smallp.tile([NSEG, R], dt.float32)
            nc.vector.tensor_scalar(cntc[:], cnt_f[:], 1.0, None, op0=Alu.max)
            recip = smallp.tile([NSEG, R], dt.float32)
            nc.vector.reciprocal(recip[:], cntc[:])

            qengs = [nc.sync, nc.scalar, nc.gpsimd][: cfg["nq"]]

            for r in range(R):
                # S[p, ti, j] = (pos[j] - p <= 128*ti), one fused DVE op over
                # two stride-0 broadcast views.
                S_all = sp.tile([TOK, KTILES, P], dt.float32, tag="sall")
                nc.vector.scalar_tensor_tensor(
                    S_all[:],
                    pos_b[:, r * P : (r + 1) * P][:, None, :]
                        .broadcast_to((TOK, KTILES, P)),
                    p_iota[:],
                    tio_p[:, :, None].broadcast_to((TOK, KTILES, P)),
                    op0=Alu.subtract,
                    op1=Alu.is_le,
                )
                # ind[p, ti, j] = S[p, ti, j] - S[p, ti, j+1]
                ind_all = indp.tile([TOK, KTILES, NSEG], dt.float32, tag="ind")
                nc.vector.tensor_tensor(
                    ind_all[:], S_all[:, :, 0:NSEG], S_all[:, :, 1:P],
                    op=Alu.subtract,
                )

                ps = psp.tile([2 * NSEG, C], dt.float32)
                xr = x_d[r].rearrange("(b k p) c -> b p k c", k=blk, p=TOK)
                for b in range(nblk):
                    xt = xp.tile([TOK, blk * C], dt.float32)
                    xt_v = xt[:].rearrange("p (k c) -> p k c", k=blk)
                    eng = qengs[(r * nblk + b) % len(qengs)]
                    eng.dma_start(xt_v, xr[b])
                    for k in range(blk):
                        ti = b * blk + k
                        rhs = xt[:, k * C : (k + 1) * C]
                        lhsT = ind_all[:, ti, :]
                        half = ti % 2
                        nc.tensor.matmul(
                            ps[half * NSEG : (half + 1) * NSEG, :], lhsT, rhs,
                            start=(ti == half), stop=(ti == KTILES - 2 + half),
                            tile_position=(0, half * NSEG),
                            skip_group_check=True,
                        )

                # y = (psum_even + psum_odd) * recip; DVE reads one PSUM
                # operand per op, so scale each half separately.
                rr = recip[:, r : r + 1]
                half_t = outp.tile([NSEG, C], dt.float32, tag="half")
                nc.vector.tensor_scalar(
                    half_t[:], ps[NSEG : 2 * NSEG, :], rr, None, op0=Alu.mult
                )
                out_t = outp.tile([NSEG, C], dt.float32, tag="out")
                nc.vector.scalar_tensor_tensor(
                    out_t[:], ps[0:NSEG, :], rr, half_t[:],
                    op0=Alu.mult, op1=Alu.add,
                )
                qengs[r % len(qengs)].dma_start(y_d[r], out_t[:])

    nc.compile()
    return nc


_PROGRAM = None


def _get_program():
    global _PROGRAM
    if _PROGRAM is None:
        _PROGRAM = build_program()
    return _PROGRAM


def kernel(x, x_pos):
    x = np.ascontiguousarray(x, dtype=np.float32)
    x_pos = np.ascontiguousarray(x_pos, dtype=np.int32)
    nc = _get_program()
    in_maps = [
        {"x": x[c * R : (c + 1) * R], "x_pos": x_pos[c * R : (c + 1) * R]}
        for c in range(NCORES)
    ]
    res = run_bass_kernel_spmd(nc, in_maps, list(range(NCORES)))
    y = np.concatenate([res.results[c]["y"] for c in range(NCORES)], axis=0)
    return y.astype(np.float32)


# revision 7
# speedup vs baseline: 1.0216x; 1.0211x over previous
"""Segment mean-pool (LocalPooling1D) Trainium2 Bass kernel.

x [32, 8192, 256] f32, x_pos [32, 65] sorted int32 boundaries -> y [32, 64, 256].
y[b, j] = mean(x[b, x_pos[b,j]:x_pos[b,j+1]]), empty segments -> 0.

Strategy: data-parallel over batch, 4 rows per core on 8 cores. The kernel is
HBM-bound (33.55 MB of x per core; ~94 us at the ~358 GB/s per-core cap), so
everything is arranged to keep the DMA engines saturated end-to-end:

- Token-interleaved x layout: token t = 512*chunk + 4*p + k lives in partition
  p, slot k. Each partition line of a chunk tile is 4 KB of contiguous DRAM,
  so every DMA descriptor moves 4 KB. A hardware DGE queue dispatches ~1
  descriptor / 6 ns, so with 4 KB descriptors TWO hw queues (sync + scalar
  engines, alternating chunks) outrun the 16 DMA engines' aggregate cap.
- The 0/1 segment indicator ind[p, ch, k, j] = (pos[j] <= t < pos[j+1]) is
  built per row in two wide DVE ops from stride-0 broadcast views (fused
  scalar_tensor_tensor compare, then a shifted subtract); pos is replicated
  across partitions with a tiny PE matmul (ones.T @ posf) so no gpsimd
  dependency sits in front of the x stream.
- Segment sums accumulate as psum += ind_tile.T @ x_slice in fp32 with even/
  odd k-slots in separate PE column groups (concurrent sub-array matmuls).
  psum_bufs=4 gives each row its own PSUM bank so late scale ops never block
  the matmul stream. Finally y = psum * 1/max(count, 1) (counts from two tiny
  transposed pos loads), written back on the otherwise-idle gpsimd queue.
"""

import os
import sys

import numpy as np

sys.path.insert(0, "/opt/trn_rl_repo")

import concourse.bacc as bacc
import concourse.bass as bass
import concourse.tile as tile
from concourse import mybir
from concourse.bass_utils import run_bass_kernel_spmd

dt = mybir.dt
Alu = mybir.AluOpType

# Problem constants (hardcoded per harness contract).
B, T, C, P = 32, 8192, 256, 65
NSEG = P - 1
NCORES = 8
R = B // NCORES          # batch rows per core
TOK = 128                # partitions (matmul contraction dim)
K = 4                    # tokens per partition line (4 KB contiguous)
CHTOK = TOK * K          # 512 tokens per chunk
CH = T // CHTOK          # 16 chunks per row

CFG = {
    "x_bufs": int(os.environ.get("KB_XBUFS", "16")),
    "ind_bufs": int(os.environ.get("KB_INDBUFS", "4")),
    "s_bufs": int(os.environ.get("KB_SBUFS", "2")),
    "psum_bufs": int(os.environ.get("KB_PSUMBUFS", "4")),
}


def build_program(cfg=CFG):
    nc = bacc.Bacc("TRN2", target_bir_lowering=False, debug=False)

    x_d = nc.dram_tensor("x", [R, T, C], dt.float32, kind="ExternalInput")
    pos_d = nc.dram_tensor("x_pos", [R, P], dt.int32, kind="ExternalInput")
    y_d = nc.dram_tensor("y", [R, NSEG, C], dt.float32, kind="ExternalOutput")

    with tile.TileContext(nc) as tc:
        with (
            tc.tile_pool(name="const", bufs=1) as constp,
            tc.tile_pool(name="xp", bufs=cfg["x_bufs"]) as xp,
            tc.tile_pool(name="sp", bufs=cfg["s_bufs"]) as sp,
            tc.tile_pool(name="indp", bufs=cfg["ind_bufs"]) as indp,
            tc.tile_pool(name="smallp", bufs=1) as smallp,
            tc.tile_pool(name="outp", bufs=2) as outp,
            tc.tile_pool(name="psp", bufs=cfg["psum_bufs"], space="PSUM") as psp,
            tc.tile_pool(name="pbp", bufs=1, space="PSUM") as pbp,
        ):
            # 512*ch + k over the (chunk, slot) axes; identical per partition.
            # All values < 2^13, exact in f32.
            tio2 = constp.tile([TOK, CH * K], dt.float32)
            nc.gpsimd.iota(tio2[:], pattern=[[CHTOK, CH], [1, K]], base=0,
                           channel_multiplier=0,
                           allow_small_or_imprecise_dtypes=True)
            # 4*p as a per-partition scalar.
            p4 = constp.tile([TOK, 1], dt.float32)
            nc.gpsimd.iota(p4[:], pattern=[[1, 1]], base=0,
                           channel_multiplier=K,
                           allow_small_or_imprecise_dtypes=True)
            ones_row = constp.tile([1, TOK], dt.float32)
            nc.vector.memset(ones_row[:], 1.0)

            # pos rows -> [1, R*P] f32; tiny dependency-free loads on the
            # gpsimd queue (which carries no x traffic).
            pos_i = smallp.tile([1, R * P], dt.int32)
            nc.gpsimd.dma_start(
                pos_i[0:1, :].rearrange("one (r p) -> one r p", r=R),
                pos_d[:, :],
            )
            posf = smallp.tile([1, R * P], dt.float32)
            nc.vector.tensor_copy(posf[:], pos_i[:])
            # Replicate to all 128 partitions via PE: ones.T @ posf (exact).
            pos_b = pbp.tile([TOK, R * P], dt.float32)
            nc.tensor.matmul(pos_b[:], ones_row[:], posf[:], start=True,
                             stop=True)

            # counts -> 1/max(cnt, 1), partition-major [NSEG, R] from two
            # transposed pos loads (DVE operands must start at partition 0).
            pos_lo = smallp.tile([NSEG, R], dt.int32)
            pos_hi = smallp.tile([NSEG, R], dt.int32)
            nc.gpsimd.dma_start(pos_lo[:],
                                pos_d[:, 0:NSEG].rearrange("r p -> p r"))
            nc.gpsimd.dma_start(pos_hi[:],
                                pos_d[:, 1:P].rearrange("r p -> p r"))
            cnt_f = smallp.tile([NSEG, R], dt.float32)
            nc.vector.tensor_tensor(cnt_f[:], pos_hi[:], pos_lo[:],
                                    op=Alu.subtract)
            cntc = smallp.tile([NSEG, R], dt.float32)
            nc.vector.tensor_scalar(cntc[:], cnt_f[:], 1.0, None, op0=Alu.max)
            recip = smallp.tile([NSEG, R], dt.float32)
            nc.vector.reciprocal(recip[:], cntc[:])

            for r in range(R):
                # S[p, m, j] = (pos[j] - 4p <= 512*(m//4) + m%4) with
                # m = (ch, k) collapsed (DVE ops allow at most 2 free dims);
                # one fused DVE op over two stride-0 broadcast views.
                S_all = sp.tile([TOK, CH * K, P], dt.float32, tag="sall")
                nc.vector.scalar_tensor_tensor(
                    S_all[:],
                    pos_b[:, r * P : (r + 1) * P][:, None, :]
                        .broadcast_to((TOK, CH * K, P)),
                    p4[:],
                    tio2[:, :, None].broadcast_to((TOK, CH * K, P)),
                    op0=Alu.subtract,
                    op1=Alu.is_le,
                )
                # ind[p, m, j] = S[p, m, j] - S[p, m, j+1]
                ind_all = indp.tile([TOK, CH * K, NSEG], dt.float32, tag="ind")
                nc.vector.tensor_tensor(
                    ind_all[:], S_all[:, :, 0:NSEG], S_all[:, :, 1:P],
                    op=Alu.subtract,
                )

                ps = psp.tile([2 * NSEG, C], dt.float32)
                xr = x_d[r].rearrange("(ch p k) c -> ch p (k c)", p=TOK, k=K)
                for ch in range(CH):
                    xt = xp.tile([TOK, K * C], dt.float32)
                    eng = nc.sync if (r * CH + ch) % 2 == 0 else nc.scalar
                    eng.dma_start(xt[:], xr[ch])
                    for k in range(K):
                        rhs = xt[:, k * C : (k + 1) * C]
                        lhsT = ind_all[:, ch * K + k, :]
                        half = k % 2
                        nc.tensor.matmul(
                            ps[half * NSEG : (half + 1) * NSEG, :], lhsT, rhs,
                            start=(ch == 0 and k == half),
                            stop=(ch == CH - 1 and k == K - 2 + half),
                            tile_position=(0, half * NSEG),
                            skip_group_check=True,
                        )

                # y = (psum_even + psum_odd) * recip; DVE reads one PSUM
                # operand per op, so scale each half separately.
                rr = recip[:, r : r + 1]
                half_t = outp.tile([NSEG, C], dt.float32, tag="half")
                nc.vector.tensor_scalar(
                    half_t[:], ps[NSEG : 2 * NSEG, :], rr, None, op0=Alu.mult
                )
                out_t = outp.tile([NSEG, C], dt.float32, tag="out")
                nc.vector.scalar_tensor_tensor(
                    out_t[:], ps[0:NSEG, :], rr, half_t[:],
                    op0=Alu.mult, op1=Alu.add,
                )
                nc.gpsimd.dma_start(y_d[r], out_t[:])

    nc.compile()
    return nc


_PROGRAM = None


def _get_program():
    global _PROGRAM
    if _PROGRAM is None:
        _PROGRAM = build_program()
    return _PROGRAM


def kernel(x, x_pos):
    x = np.ascontiguousarray(x, dtype=np.float32)
    x_pos = np.ascontiguousarray(x_pos, dtype=np.int32)
    nc = _get_program()
    in_maps = [
        {"x": x[c * R : (c + 1) * R], "x_pos": x_pos[c * R : (c + 1) * R]}
        for c in range(NCORES)
    ]
    res = run_bass_kernel_spmd(nc, in_maps, list(range(NCORES)))
    y = np.concatenate([res.results[c]["y"] for c in range(NCORES)], axis=0)
    return y.astype(np.float32)
